# revision 26
# baseline (speedup 1.0000x reference)
"""Trainium2 Bass kernel for a 4x2048x768 no-scale no-mask attention block.

Sharding: 8 cores = 4 batches x 2 query-halves. Each core computes H = x A^T
(A = Wq^T Wk) over the full (rolled) 2048-key sequence, attention for its 1024
queries, and a fused PV/out projection. SPMD-identical across cores: the host
rolls each core's copy of x along the sequence axis so the core's own queries
occupy columns 0:1024 (softmax is invariant to key permutation).

Host-side weight algebra (exact):
  scores S[i,j] = x_i A x_j^T + w[j] + (terms constant in j, dropped)
      A = Wq^T Wk,  w = x (Wk^T bq)  [w computed on host, fed as exp bias]
  V/out fusion: because softmax rows sum to 1,
      out = P x (Wo Wv)^T + (bo + Wo bv)
  so the V projection disappears entirely: the kernel computes y' = P x
  against a token-major bf16 copy of x and projects with B = Wo Wv.

Device pipeline (per core):
  P1 HT[e, s] = (x A^T)^T via 4 column sweeps (d-contraction in PSUM).
  P2 per 512-query block: 16 S^T tiles [keys,128 x queries,512] -> ACT exp
     (bias w) -> bf16 P^T tiles; Z row via serial DVE accumulation of the 16
     exp tiles + one ones-column matmul; PV' h-major (stationary = bf16
     token-major x slices) with the out-projection (moving B^T) injected
     progressively one h behind, so the post-attention tail is ~3us.
  Normalization is deferred: out = (y' B^T) * (1/Z) + boe.
All big matmuls are f32r (full PE rate) except PV' which is bf16 x bf16.
DMA launches alternate between the sync and scalar engines; a few warm-up
matmuls on the boe row ramp the PE p-state while the first weights stream in.
"""

import sys

if "/opt/trn_rl_repo" not in sys.path:
    sys.path.insert(0, "/opt/trn_rl_repo")

import numpy as np
import ml_dtypes

B = 4
S = 2048
D = 768
DT = D // 128  # 6 feature tiles
QH = 1024  # queries per core
NCORES = 8
NJ = S // 128  # 16 key tiles
NJC = S // 512  # 4 HT column sweeps

_CACHE = {}
last_results = None  # BassKernelResults of the most recent run (for test harness)


def _build_nc():
    if "nc" in _CACHE:
        return _CACHE["nc"]

    from concourse import bacc, mybir
    import concourse.tile as tile

    f32 = mybir.dt.float32
    f32r = mybir.dt.float32r
    bf16 = mybir.dt.bfloat16
    AF = mybir.ActivationFunctionType

    nc = bacc.Bacc("TRN2", target_bir_lowering=False, debug=False)

    def dram(name, shape, kind, dt=f32):
        return nc.dram_tensor(name, list(shape), dt, kind=kind).ap()

    xT = dram("xT", (DT, 128, S), "ExternalInput", f32r)  # x[b].T rolled, d-tiled
    waT = dram("waT", (DT, 128, D), "ExternalInput", f32r)  # (Wq^T Wk)^T tiles
    wbT = dram("wbT", (128, DT * D), "ExternalInput", f32r)  # (Wo Wv)^T, partition-major
    xkT = dram("xkT", (128, NJ * D), "ExternalInput", bf16)  # token-major x, partition-major
    wcolT = dram("wcolT", (128, NJ), "ExternalInput")  # x (Wk^T bq), tiled
    boe = dram("boe", (1, D), "ExternalInput", f32r)  # bo + wo @ bv
    out = dram("out", (QH, D), "ExternalOutput")

    with tile.TileContext(nc) as tc:
        # ---- long-lived constants (left side) ----
        consts = tc.alloc_tile_pool(name="consts", bufs=1, side="left")
        ones_f = consts.tile([128, 8], f32, tag="ones_f", name="ones_f")
        nc.vector.memset(ones_f, 1.0)
        ones = consts.tile([128, 8], f32r, tag="ones", name="ones")
        nc.vector.tensor_copy(ones, ones_f)
        onesr_f = consts.tile([1, 128], f32, tag="onesr_f", name="onesr_f")
        nc.vector.memset(onesr_f, 1.0)
        onesr = consts.tile([1, 128], f32r, tag="onesr", name="onesr")
        nc.vector.tensor_copy(onesr, onesr_f)
        boe_sb = consts.tile([1, D], f32r, tag="boe", name="boe_sb")
        wcol = consts.tile([128, NJ], f32, tag="wcol", name="wcol")
        boe_bc = consts.tile([128, D], f32, tag="boe_bc", name="boe_bc")
        warm_f = consts.tile([1, 512], f32, tag="warm_f", name="warm_f")
        nc.vector.memset(warm_f, 1.0)
        warm_r = consts.tile([1, 512], f32r, tag="warm_r", name="warm_r")
        nc.vector.tensor_copy(warm_r, warm_f)

        # ---- phase inputs ----
        xpool = tc.alloc_tile_pool(name="xpool", bufs=1, side="right")
        xkpool = tc.alloc_tile_pool(name="xkpool", bufs=1, side="right")
        # wapool is top of the right-side pool stack: released after HT, its
        # space is reused by wbpool (the out-projection weights).
        wapool = tc.alloc_tile_pool(name="wapool", bufs=1, side="right")

        xt = [
            xpool.tile([128, S], f32r, tag=f"xt{d}", name=f"xt{d}") for d in range(DT)
        ]
        wa = [
            wapool.tile([128, D], f32r, tag=f"wa{d}", name=f"wa{d}") for d in range(DT)
        ]
        xtok = xkpool.tile([128, NJ * D], bf16, tag="xtok", name="xtok")

        # Critical-path DMAs: the first HT sweep needs all six (wa, x jc0)
        # pairs, so interleave them across both launch engines in d order.
        # (One launch per tile: transfers stripe across all 16 DMA engines.)
        nc.sync.dma_start(out=boe_sb, in_=boe)
        nc.scalar.dma_start(out=wcol, in_=wcolT)
        for d in range(DT):
            ea, eb = (nc.sync, nc.scalar) if d % 2 == 0 else (nc.scalar, nc.sync)
            ea.dma_start(out=wa[d], in_=waT[d])
            eb.dma_start(out=xt[d][:, 0:512], in_=xT[d][:, 0:512])
        # Remaining x columns and the token-major bf16 copy. Bulk rides in as
        # FEW launches as possible: the hardware queues round-robin service
        # across pending DMAs, so every extra pending bulk launch steals
        # bandwidth share from the critical first-sweep transfers above.
        for d in range(DT):
            eng = nc.sync if d % 2 == 0 else nc.scalar
            eng.dma_start(out=xt[d][:, 512:S], in_=xT[d][:, 512:S])
        nc.scalar.dma_start(out=xtok, in_=xkT)

        # ---- pools for HT + attention ----
        hpool = tc.alloc_tile_pool(name="hpool", bufs=1, side="left")
        ht = [
            hpool.tile([128, S], f32r, tag=f"ht{h}", name=f"ht{h}") for h in range(DT)
        ]
        expool = tc.alloc_tile_pool(name="expool", bufs=16, side="left")
        zpool = tc.alloc_tile_pool(name="zpool", bufs=2, side="left")
        ytpool = tc.alloc_tile_pool(name="ytpool", bufs=1, side="left")
        outpool = tc.alloc_tile_pool(name="outpool", bufs=4, side="left")
        yt = ytpool.tile([128, DT * 512], f32r, tag="yt", name="yt")

        # Block state: per query block, the 16 exp tiles and the running DVE
        # sum feeding Z.
        ex_blk = {0: [], 1: []}
        acc_blk = {}

        def emit_st(pool, tag, bufs, ib, j):
            """One S^T key tile for query block ib: 6 matmuls + exp + Z add."""
            io = ib * 512
            stp = pool.tile([128, 512], f32, tag=tag, name=f"st{ib}_{j}", bufs=bufs)
            for d in range(DT):
                nc.tensor.matmul(
                    stp,
                    ht[d][:, j * 128 : (j + 1) * 128],
                    xt[d][:, io : io + 512],
                    start=(d == 0),
                    stop=(d == DT - 1),
                )
            e = expool.tile([128, 512], bf16, tag="ex", name=f"ex{ib}_{j}")
            nc.scalar.activation(e, stp, AF.Exp, bias=wcol[:, j : j + 1])
            ex = ex_blk[ib]
            ex.append(e)
            if j == 1:
                acc_blk[ib] = zpool.tile(
                    [128, 512], f32, tag="acc", name=f"acc{ib}_1"
                )
                nc.vector.tensor_add(acc_blk[ib], ex[0], ex[1])
            elif j > 1:
                dt_j = f32r if j == NJ - 1 else f32
                nxt = zpool.tile(
                    [128, 512], dt_j, tag="accr" if j == NJ - 1 else "acc",
                    name=f"acc{ib}_{j}",
                )
                nc.vector.tensor_add(nxt, acc_blk[ib], e)
                acc_blk[ib] = nxt

        # ---- P1: warm-up + boe broadcast + HT sweeps, with block-0's S^T
        # tiles interleaved after each sweep (they need no new DMA bytes, so
        # they soak up the first sweep's DMA-bound stalls). ----
        paA = tc.alloc_tile_pool(name="paA", bufs=1, space="PSUM")

        # Warm-up: rank-1 matmuls on a memset row (no DMA dependency) keep the
        # PE busy and ramp its p-state while the first weight tiles stream in.
        wj = paA.tile([128, 512], f32, tag="stA", name="warm", bufs=2)
        for i in range(6):
            nc.tensor.matmul(wj, onesr, warm_r, start=True, stop=True)
        # boe broadcast across partitions (rank-1 matmuls on the boe row).
        nc.tensor.matmul(wj, onesr, boe_sb[0:1, 0:512], start=True, stop=True)
        nc.vector.tensor_copy(boe_bc[:, 0:512], wj)
        wj2 = paA.tile([128, 256], f32, tag="stA", name="warm2", bufs=2)
        nc.tensor.matmul(wj2, onesr, boe_sb[0:1, 512:768], start=True, stop=True)
        nc.vector.tensor_copy(boe_bc[:, 512:768], wj2)

        for jc in range(NJC):
            hps = [
                paA.tile([128, 512], f32, tag="hps", name=f"hps{jc}_{h}", bufs=6)
                for h in range(DT)
            ]
            for d in range(DT):
                for h in range(DT):
                    nc.tensor.matmul(
                        hps[h],
                        wa[d][:, h * 128 : (h + 1) * 128],
                        xt[d][:, jc * 512 : (jc + 1) * 512],
                        start=(d == 0),
                        stop=(d == DT - 1),
                    )
                    # Drain each h-bank as soon as its accumulation closes so
                    # the next sweep's banks free up behind the PE.
                    if d == DT - 1:
                        nc.scalar.activation(
                            ht[h][:, jc * 512 : (jc + 1) * 512], hps[h], AF.Copy
                        )
            for j in range(jc * 4, jc * 4 + 4):
                emit_st(paA, "stA", 2, 0, j)
        paA.release()
        wapool.release()

        # ---- out-projection weights: loaded into the space wa vacated ----
        wbpool = tc.alloc_tile_pool(name="wbpool", bufs=1, side="right")
        wb_all = wbpool.tile([128, DT * D], f32r, tag="wb", name="wb_all")
        nc.sync.dma_start(out=wb_all, in_=wbT)
        wb = [wb_all[:, h * D : (h + 1) * D] for h in range(DT)]

        # ---- P2: attention + fused out-projection, per 512-query block ----
        # One PSUM pool, 8 banks: sp x4 (block-1 S^T tiles, Z, PV' rotate
        # through one ring), opa x4 (progressive out-projection accumulators).
        paB = tc.alloc_tile_pool(name="paB", bufs=1, space="PSUM")

        for ib in range(QH // 512):
            io = ib * 512
            if ib > 0:
                for j in range(NJ):
                    emit_st(paB, "sp", 4, ib, j)
            ex = ex_blk[ib]
            acc = acc_blk[ib]

            # PV' h-major with the out-projection injected one h behind.
            opa = [
                paB.tile([128, 512], f32, tag="opa", name=f"opa{ib}_{t}", bufs=4)
                for t in range(4)
            ]

            def out_proj(h, opa=opa):
                for t in range(4):
                    lhs = yt[:, h * 512 + t * 128 : h * 512 + (t + 1) * 128]
                    nc.tensor.matmul(
                        opa[t], lhs, wb[h][:, 0:512], start=(h == 0), stop=(h == DT - 1)
                    )

            rz = None
            for h in range(DT):
                pvp = paB.tile([128, 512], f32, tag="sp", name=f"pv{ib}_{h}", bufs=4)
                for j in range(NJ):
                    nc.tensor.matmul(
                        pvp,
                        xtok[:, j * D + h * 128 : j * D + (h + 1) * 128],
                        ex[j],
                        start=(j == 0),
                        stop=(j == NJ - 1),
                    )
                if h == 0:
                    # Z row -> reciprocal column, emitted right after PV h=0 so
                    # the PE flows from the last S^T tile straight into PV.
                    zp = paB.tile([128, 512], f32, tag="sp", name=f"zp{ib}", bufs=4)
                    nc.tensor.matmul(zp[0:8, :], ones, acc, start=True, stop=True)
                nc.vector.tensor_copy(yt[:, h * 512 : (h + 1) * 512], pvp)
                if h == 0:
                    z_f = zpool.tile([1, 512], f32, tag="zf", name=f"z_f{ib}")
                    nc.vector.tensor_copy(z_f, zp[0:1, :])
                    zcol = zpool.tile([128, 4], f32, tag="zc", name=f"zcol{ib}")
                    for t in range(4):
                        nc.sync.dma_start(
                            out=zcol[:, t : t + 1],
                            in_=z_f[0:1, t * 128 : (t + 1) * 128],
                        )
                    rz = zpool.tile([128, 4], f32, tag="rz", name=f"rz{ib}")
                    nc.vector.reciprocal(rz, zcol)
                else:
                    out_proj(h - 1)
            out_proj(DT - 1)

            # Tail: scale the 0:512 columns by 1/Z and add boe now (single
            # fused DVE op per tile, overlapping the opb matmuls below), then
            # accumulate the 512:768 columns per query-tile in sp-ring banks.
            osb = [
                outpool.tile([128, D], f32, tag="ot", name=f"osb{ib}_{t}")
                for t in range(4)
            ]
            # The 0:512 columns are complete once out_proj(5) retires: scale,
            # bias, and STORE them now so those transfers overlap the opb
            # matmuls; only the thin 512:768 store remains at the very end.
            for t in range(4):
                nc.vector.scalar_tensor_tensor(
                    osb[t][:, 0:512],
                    opa[t][:, 0:512],
                    rz[:, t : t + 1],
                    boe_bc[:, 0:512],
                    mybir.AluOpType.mult,
                    mybir.AluOpType.add,
                )
                ro = io + t * 128
                eng = nc.sync if t % 2 == 0 else nc.scalar
                eng.dma_start(
                    out=out[ro : ro + 128, 0:512], in_=osb[t][:, 0:512]
                )
            for t in range(4):
                opb = paB.tile([128, 512], f32, tag="sp", name=f"opb{ib}_{t}", bufs=4)
                for h in range(DT):
                    nc.tensor.matmul(
                        opb[:, 0:256],
                        yt[:, h * 512 + t * 128 : h * 512 + (t + 1) * 128],
                        wb[h][:, 512:768],
                        start=(h == 0),
                        stop=(h == DT - 1),
                    )
                nc.vector.scalar_tensor_tensor(
                    osb[t][:, 512:768],
                    opb[:, 0:256],
                    rz[:, t : t + 1],
                    boe_bc[:, 512:768],
                    mybir.AluOpType.mult,
                    mybir.AluOpType.add,
                )
                ro = io + t * 128
                eng = nc.scalar if t % 2 == 0 else nc.sync
                eng.dma_start(
                    out=out[ro : ro + 128, 512:768], in_=osb[t][:, 512:768]
                )

        for p in (paB, outpool, ytpool, zpool, expool, wbpool, xkpool,
                  hpool, xpool, consts):
            p.release()

    nc.compile()
    _CACHE["nc"] = nc
    return nc


def _shard_inputs(x, wq, bq, wk, bk, wv, bv, wo, bo):
    """Build the 8 per-core input maps (host-side layout + weight algebra)."""
    f = np.float32
    f8 = np.float64
    bf = ml_dtypes.bfloat16
    x = np.asarray(x, f)
    wq, wk, wv, wo = (np.asarray(a, f) for a in (wq, wk, wv, wo))
    bq, bk, bv, bo = (np.asarray(a, f) for a in (bq, bk, bv, bo))

    def wtiles(w):  # [out, in] -> [in-tile, 128, out]
        return np.ascontiguousarray(np.asarray(w, f).T).reshape(DT, 128, D)

    A = (wq.astype(f8).T @ wk.astype(f8)).astype(f)  # H = x @ A.T
    Bm = (wo.astype(f8) @ wv.astype(f8)).astype(f)  # out = (P x) @ Bm.T + boe
    wkbq_col = wk.astype(f8).T @ bq.astype(f8)  # [768]
    shared = {
        "waT": wtiles(A),
        # [128, DT*D], partition-major so it loads as a single DMA launch
        "wbT": np.ascontiguousarray(
            wtiles(Bm).transpose(1, 0, 2).reshape(128, DT * D)
        ),
        "boe": (bo.astype(f8) + wo.astype(f8) @ bv.astype(f8)).astype(f).reshape(1, D),
    }
    in_maps = []
    for c in range(NCORES):
        b, half = c // 2, c % 2
        xr = x[b]  # [S, D] token-major
        if half:
            xr = np.concatenate([xr[QH:], xr[:QH]], axis=0)
        m = dict(shared)
        m["xT"] = np.ascontiguousarray(xr.T).reshape(DT, 128, S)
        # [128, NJ*D]: xkT[p, j*D + e] = xr[j*128 + p, e]; single DMA launch
        m["xkT"] = np.ascontiguousarray(
            xr.astype(bf).reshape(NJ, 128, D).transpose(1, 0, 2).reshape(128, NJ * D)
        )
        w = (xr.astype(f8) @ wkbq_col).astype(f)  # [S]
        m["wcolT"] = np.ascontiguousarray(w.reshape(NJ, 128).T)
        in_maps.append(m)
    return in_maps


def kernel(x, wq, bq, wk, bk, wv, bv, wo, bo, trace=False, trace_kwargs=None):
    global last_results
    from concourse.bass_utils import run_bass_kernel_spmd

    nc = _build_nc()
    in_maps = _shard_inputs(x, wq, bq, wk, bk, wv, bv, wo, bo)
    res = run_bass_kernel_spmd(
        nc,
        in_maps,
        core_ids=list(range(NCORES)),
        trace=trace,
        **(trace_kwargs or {}),
    )
    last_results = res
    out = np.empty((B, S, D), np.float32)
    for c in range(NCORES):
        b, half = c // 2, c % 2
        out[b, half * QH : (half + 1) * QH, :] = res.results[c]["out"]
    return out
